# revision 42
# baseline (speedup 1.0000x reference)
"""Trainium2 Bass kernel for AngleFreqEnhance (histogram_binning).

Sharding: 8 cores = 4 samples x 2 output-channel-halves, no collectives.
Each core streams both input-channel halves of its sample (16MB), computes
the full 16-plane 1x1-conv projection via PSUM-accumulated matmuls, runs the
FFT / histogram / peak-search / gain / iFFT pipeline (redundant within the
pair; matmul-DFT on the PE), and produces its half of the output channels
(proj_out with the residual fused into the PSUM eviction, 8MB out).

FFT = matmul with precomputed shifted-DFT matrices (fftshift folded in).
Histogram = matmul with a precomputed one-hot (bin x pixel) bf16 table.
Peak search = vector-engine reductions on the [1,180] histogram.
DMAs are consolidated into few large transfers; elementwise work is spread
across DVE / ACT / GpSimd (GpSimd never touches PSUM).
"""
import os
import sys
import numpy as np

for _p in ("/opt/trn_rl_repo", "/root/.axon_site/_ro/trn_rl_repo"):
    if os.path.isdir(_p) and _p not in sys.path:
        sys.path.insert(0, _p)

N_BINS = 180
PI = float(np.pi)
BW = float(np.radians(15.0))
HIGH_RATIO = 0.3
ALPHA = 1.2
BETA = 0.8

_CACHE = {}


def _build_tables():
    import jax
    cpu = jax.devices('cpu')[0]
    import jax.numpy as jnp
    with jax.default_device(cpu):
        yy, xx = jnp.meshgrid(jnp.arange(128), jnp.arange(128), indexing='ij')
        dy = (yy - 64).astype(jnp.float32)
        dx = (xx - 64).astype(jnp.float32)
        r = jnp.sqrt(dy * dy + dx * dx)
        theta = np.array(jnp.arctan2(dy, dx) + PI, dtype=np.float32)
        theta_mod = np.array(jnp.asarray(theta) % PI, dtype=np.float32)
        edges = np.array(jnp.linspace(0.0, PI, N_BINS + 1), dtype=np.float32)
        bin_idx = np.array(
            jnp.clip(jnp.searchsorted(jnp.asarray(edges), jnp.asarray(theta_mod).reshape(-1), side='left') - 1,
                     0, N_BINS - 1)).reshape(128, 128)
        high = np.array(r > HIGH_RATIO * 64.0)
        centers = np.array((jnp.asarray(edges)[:-1] + jnp.asarray(edges)[1:]) * 0.5, dtype=np.float32)

    k = np.arange(128)
    Wm = np.exp(-2j * np.pi * np.outer(k, k) / 128) / np.sqrt(128)
    S = np.zeros((128, 128))
    S[np.arange(128), (np.arange(128) + 64) % 128] = 1
    Ds = S @ Wm                   # fftshift(fft2(X)) = Ds @ X @ Ds.T
    Ci = np.conj(Wm) @ S.T        # ifft2(ifftshift(G)) = Ci @ G @ Ci.T

    import ml_dtypes
    bt = np.zeros((128, 128 * N_BINS), dtype=np.float32)
    for c in range(128):
        col_bins = bin_idx[c, :]
        col_high = high[c, :].astype(np.float32)
        bt[np.arange(128), c * N_BINS + col_bins] = col_high

    tabs = np.concatenate([
        Ds.real.T.astype(np.float32), Ds.imag.T.astype(np.float32),
        Ci.real.T.astype(np.float32), Ci.imag.T.astype(np.float32),
        theta.T,
    ], axis=1)                    # [128, 640]
    vecs = np.concatenate([
        np.arange(N_BINS, dtype=np.float32).reshape(1, -1),
        centers.reshape(1, -1),
    ], axis=1)                    # [1, 360]
    return dict(
        tabs=np.ascontiguousarray(tabs),
        vecs=np.ascontiguousarray(vecs),
        highT=np.ascontiguousarray(high.T.astype(np.uint8)),
        btab=bt.astype(ml_dtypes.bfloat16),
    )


def _build_graph():
    from concourse import bacc, tile, mybir
    from concourse.mybir import AluOpType as alu
    from concourse.mybir import ActivationFunctionType as act

    dt = mybir.dt
    f32 = dt.float32
    f32r = dt.float32r
    bf16 = dt.bfloat16

    nc = bacc.Bacc("TRN2", target_bir_lowering=False, debug=False, num_devices=8)

    xh_d = nc.dram_tensor("xh", [128, 16384], f32r, kind="ExternalInput").ap()
    xo_d = nc.dram_tensor("xo", [128, 16384], f32r, kind="ExternalInput").ap()
    winT_d = nc.dram_tensor("winT", [128, 16], f32r, kind="ExternalInput").ap()
    winO_d = nc.dram_tensor("winO", [128, 16], f32r, kind="ExternalInput").ap()
    woutT_d = nc.dram_tensor("woutT", [16, 128], f32r, kind="ExternalInput").ap()
    tabs_d = nc.dram_tensor("tabs", [128, 640], f32r, kind="ExternalInput").ap()
    highT_d = nc.dram_tensor("highT", [128, 128], dt.uint8, kind="ExternalInput").ap()
    btab_d = nc.dram_tensor("btab", [128, 128 * N_BINS], bf16, kind="ExternalInput").ap()
    vecs_d = nc.dram_tensor("vecs", [1, 360], f32, kind="ExternalInput").ap()
    out_d = nc.dram_tensor("out", [128, 16384], f32, kind="ExternalOutput").ap()

    RG = [[0, 1], [2, 3], [4, 5], [6, 7]]
    NEG = -1.0e30
    BIG = 1.0e30
    f32PI = float(np.float32(PI))
    f32BW = float(np.float32(BW))

    with tile.TileContext(nc) as tc:
        with (
            tc.tile_pool(name="dram", bufs=1, space="DRAM") as dram,
            tc.tile_pool(name="const", bufs=1) as cst,
            tc.tile_pool(name="xhp", bufs=1) as xhp,
            tc.tile_pool(name="planes", bufs=2) as plp,
            tc.tile_pool(name="ft", bufs=1) as ftp,
            tc.tile_pool(name="work", bufs=3) as wk,
            tc.tile_pool(name="small", bufs=1) as sm,
            tc.tile_pool(name="psum", bufs=6, space="PSUM") as ps,
            tc.tile_pool(name="psum1", bufs=2, space="PSUM") as ps1,
        ):
            xp_loc = dram.tile([16, 16384], f32)
            xe = dram.tile([16, 16384], f32)

            # ---- inputs into SBUF (few large DMAs; xh first, btab on ACT queue) ----
            winT = cst.tile([128, 16], f32r)
            nc.sync.dma_start(out=winT[:], in_=winT_d[:])
            winO = cst.tile([128, 16], f32r)
            nc.sync.dma_start(out=winO[:], in_=winO_d[:])
            xh = xhp.tile([128, 16384], f32r)
            for q in range(4):
                nc.sync.dma_start(out=xh[:, q * 4096:(q + 1) * 4096],
                                  in_=xh_d[:, q * 4096:(q + 1) * 4096])
            woutT = cst.tile([16, 128], f32r)
            nc.sync.dma_start(out=woutT[:], in_=woutT_d[:])
            tabs = cst.tile([128, 640], f32r)
            nc.sync.dma_start(out=tabs[:], in_=tabs_d[:])
            highT = cst.tile([128, 128], dt.uint8)
            nc.sync.dma_start(out=highT[:], in_=highT_d[:])
            vecs = cst.tile([1, 360], f32)
            nc.sync.dma_start(out=vecs[:], in_=vecs_d[:])

            rhs_ds = tabs[:, 0:256]
            rhs_ci = tabs[:, 256:512]
            dsrT = tabs[:, 0:128]
            dsiT = tabs[:, 128:256]
            cirT = tabs[:, 256:384]
            ciiT = tabs[:, 384:512]
            thetaT = tabs[:, 512:640].bitcast(f32)
            iota = vecs[:, 0:N_BINS]
            cent = vecs[:, N_BINS:2 * N_BINS]

            ones_r = cst.tile([1, 128], f32)
            nc.vector.memset(ones_r[:], 1.0)
            ones2 = cst.tile([2, 1], f32)
            nc.vector.memset(ones2[:], 1.0)
            one_m = cst.tile([128, 128], f32)
            nc.vector.memset(one_m[:], 1.0)
            negt = cst.tile([1, N_BINS], f32)
            nc.vector.memset(negt[:], NEG)
            bigt = cst.tile([1, N_BINS], f32)
            nc.vector.memset(bigt[:], BIG)

            # ---- proj_in: full x_proj, both channel halves accumulated in PSUM ----
            for q in range(8):
                xoq = wk.tile([128, 2048], f32r, tag="xoq", bufs=2)
                nc.scalar.dma_start(out=xoq[:], in_=xo_d[:, q * 2048:(q + 1) * 2048])
                xpq = wk.tile([16, 2048], f32, tag="xpq", bufs=2)
                for k in range(4):
                    c = q * 2048 + k * 512
                    pA = ps.tile([16, 512], f32, tag="mm")
                    nc.tensor.matmul(pA[:], lhsT=winT[:], rhs=xh[:, c:c + 512],
                                     start=True, stop=False)
                    nc.tensor.matmul(pA[:], lhsT=winO[:], rhs=xoq[:, k * 512:(k + 1) * 512],
                                     start=False, stop=True)
                    nc.vector.tensor_copy(out=xpq[:, k * 512:(k + 1) * 512], in_=pA[:])
                nc.sync.dma_start(out=xp_loc[:, q * 2048:(q + 1) * 2048], in_=xpq[:])

            # ---- forward DFT per plane + magnitude accumulation ----
            wsum = sm.tile([128, 128], f32)
            nc.vector.memset(wsum[:], 0.0)
            ft_tiles = []
            for g in range(4):
                pg = plp.tile([128, 512], f32r, tag="pgrp")
                nc.sync.dma_start(
                    out=pg[:].rearrange("h (p w) -> h p w", p=4),
                    in_=xp_loc[4 * g:4 * g + 4, :].rearrange("p (h w) -> h p w", h=128).bitcast(f32r))
                for j in range(4):
                    m = 4 * g + j
                    P = pg[:, j * 128:(j + 1) * 128]
                    s1p = ps.tile([128, 256], f32, tag="mm")
                    nc.tensor.matmul(s1p[:], lhsT=P, rhs=rhs_ds, start=True, stop=True)
                    s1 = wk.tile([128, 256], f32r, tag="s1")
                    nc.vector.tensor_copy(out=s1[:], in_=s1p[:])
                    m2a = ps.tile([128, 256], f32, tag="mm")
                    nc.tensor.matmul(m2a[:], lhsT=dsrT, rhs=s1[:], start=True, stop=True)
                    m2b = ps.tile([128, 256], f32, tag="mm")
                    nc.tensor.matmul(m2b[:], lhsT=dsiT, rhs=s1[:], start=True, stop=True)
                    m2bs = wk.tile([128, 256], f32, tag="m2bs")
                    nc.vector.tensor_copy(out=m2bs[:], in_=m2b[:])
                    ft = ftp.tile([128, 256], f32, tag=f"ft{m}")
                    ft_tiles.append(ft)
                    nc.vector.tensor_tensor(out=ft[:, 0:128], in0=m2a[:, 0:128],
                                            in1=m2bs[:, 128:256], op=alu.subtract)
                    nc.vector.tensor_tensor(out=ft[:, 128:256], in0=m2a[:, 128:256],
                                            in1=m2bs[:, 0:128], op=alu.add)
                    sq1 = wk.tile([128, 128], f32, tag="sq1")
                    nc.scalar.square(out=sq1[:], in_=ft[:, 0:128])
                    sq2 = wk.tile([128, 128], f32, tag="sq2")
                    nc.scalar.square(out=sq2[:], in_=ft[:, 128:256])
                    nc.gpsimd.tensor_tensor(out=sq1[:], in0=sq1[:], in1=sq2[:], op=alu.add)
                    nc.scalar.sqrt(out=sq2[:], in_=sq1[:])
                    nc.gpsimd.tensor_tensor(out=wsum[:], in0=wsum[:], in1=sq2[:], op=alu.add)

            # ---- histogram: 128 accumulating matmuls against bf16 one-hot table ----
            wsum_bf = sm.tile([128, 128], bf16)
            nc.vector.tensor_copy(out=wsum_bf[:], in_=wsum[:])
            hp = ps1.tile([2, 2 * N_BINS], f32, tag="hp", bufs=1)
            for q in range(8):
                btq = wk.tile([128, 16 * N_BINS], bf16, tag="btq", bufs=2)
                nc.scalar.dma_start(out=btq[:], in_=btab_d[:, q * 16 * N_BINS:(q + 1) * 16 * N_BINS])
                for k in range(8):
                    c = q * 8 + k
                    nc.tensor.matmul(hp[:], lhsT=wsum_bf[:, 2 * c:2 * c + 2],
                                     rhs=btq[:, 2 * k * N_BINS:(2 * k + 2) * N_BINS],
                                     start=(c == 0), stop=(c == 63))

            # ---- smoothing + peak selection on [1,180] ----
            hsb = sm.tile([2, 2 * N_BINS], f32)
            nc.vector.tensor_copy(out=hsb[:], in_=hp[:])
            hodd = sm.tile([1, N_BINS], f32)
            nc.sync.dma_start(out=hodd[:], in_=hsb[1:2, N_BINS:2 * N_BINS])
            h0 = sm.tile([1, N_BINS], f32)
            nc.vector.tensor_tensor(out=h0[:], in0=hsb[0:1, 0:N_BINS],
                                    in1=hodd[:], op=alu.add)
            t182 = sm.tile([1, 182], f32)
            nc.vector.memset(t182[:], 0.0)
            nc.vector.tensor_copy(out=t182[:, 1:181], in_=h0[:])
            s1q = sm.tile([1, N_BINS], f32, tag="pk", bufs=5)
            nc.vector.tensor_tensor(out=s1q[:], in0=t182[:, 0:180], in1=t182[:, 2:182], op=alu.add)
            e = sm.tile([1, N_BINS], f32)
            nc.vector.tensor_single_scalar(out=s1q[:], in_=s1q[:], scalar=0.25, op=alu.mult)
            nc.vector.scalar_tensor_tensor(out=e[:], in0=t182[:, 1:181], scalar=0.5,
                                           in1=s1q[:], op0=alu.mult, op1=alu.add)
            esum = sm.tile([1, 1], f32)
            nc.vector.reduce_sum(esum[:], e[:], axis=mybir.AxisListType.X)
            mean_t = sm.tile([1, 1], f32)
            nc.scalar.mul(out=mean_t[:], in_=esum[:], mul=1.0 / N_BINS)
            ew = sm.tile([1, 182], f32)
            nc.vector.tensor_copy(out=ew[:, 1:181], in_=e[:])
            nc.vector.tensor_copy(out=ew[:, 0:1], in_=e[:, 179:180])
            nc.vector.tensor_copy(out=ew[:, 181:182], in_=e[:, 0:1])
            mgm = sm.tile([1, N_BINS], f32, tag="pk", bufs=5)
            nc.vector.tensor_scalar(out=mgm[:], in0=e[:], scalar1=mean_t[:], scalar2=None, op0=alu.is_gt)
            mgl = sm.tile([1, N_BINS], f32, tag="pk", bufs=5)
            nc.vector.tensor_tensor(out=mgl[:], in0=e[:], in1=ew[:, 0:180], op=alu.is_gt)
            mgr = sm.tile([1, N_BINS], f32, tag="pk", bufs=5)
            nc.vector.tensor_tensor(out=mgr[:], in0=e[:], in1=ew[:, 2:182], op=alu.is_gt)
            ispk = sm.tile([1, N_BINS], f32, tag="pk", bufs=5)
            nc.vector.tensor_tensor(out=ispk[:], in0=mgm[:], in1=mgl[:], op=alu.mult)
            ispk2 = sm.tile([1, N_BINS], f32, tag="pk", bufs=5)
            nc.vector.tensor_tensor(out=ispk2[:], in0=ispk[:], in1=mgr[:], op=alu.mult)
            ispk2u = sm.tile([1, N_BINS], dt.uint8)
            nc.vector.tensor_copy(out=ispk2u[:], in_=ispk2[:])
            npk = sm.tile([1, 1], f32)
            nc.vector.reduce_sum(npk[:], ispk2[:], axis=mybir.AxisListType.X)
            masked = sm.tile([1, N_BINS], f32, tag="pk", bufs=5)
            nc.vector.select(out=masked[:], mask=ispk2u[:], on_true=e[:], on_false=negt[:])
            m1v = sm.tile([1, 1], f32)
            nc.vector.reduce_max(m1v[:], masked[:], axis=mybir.AxisListType.X)
            eq1 = sm.tile([1, N_BINS], dt.uint8)
            nc.vector.tensor_scalar(out=eq1[:], in0=masked[:], scalar1=m1v[:], scalar2=None, op0=alu.is_equal)
            sel1 = sm.tile([1, N_BINS], f32, tag="pk", bufs=5)
            nc.vector.select(out=sel1[:], mask=eq1[:], on_true=iota, on_false=bigt[:])
            idx1 = sm.tile([1, 1], f32)
            nc.vector.tensor_reduce(idx1[:], sel1[:], axis=mybir.AxisListType.X, op=alu.min)
            mi1 = sm.tile([1, N_BINS], dt.uint8)
            nc.vector.tensor_scalar(out=mi1[:], in0=iota, scalar1=idx1[:], scalar2=None, op0=alu.is_equal)
            masked2 = sm.tile([1, N_BINS], f32, tag="pk", bufs=5)
            nc.vector.select(out=masked2[:], mask=mi1[:], on_true=negt[:], on_false=masked[:])
            m2v = sm.tile([1, 1], f32)
            nc.vector.reduce_max(m2v[:], masked2[:], axis=mybir.AxisListType.X)
            eq2 = sm.tile([1, N_BINS], dt.uint8)
            nc.vector.tensor_scalar(out=eq2[:], in0=masked2[:], scalar1=m2v[:], scalar2=None, op0=alu.is_equal)
            sel2 = sm.tile([1, N_BINS], f32, tag="pk", bufs=5)
            nc.vector.select(out=sel2[:], mask=eq2[:], on_true=iota, on_false=bigt[:])
            idx2 = sm.tile([1, 1], f32)
            nc.vector.tensor_reduce(idx2[:], sel2[:], axis=mybir.AxisListType.X, op=alu.min)
            m0v = sm.tile([1, 1], f32)
            nc.vector.reduce_max(m0v[:], e[:], axis=mybir.AxisListType.X)
            eq0 = sm.tile([1, N_BINS], dt.uint8)
            nc.vector.tensor_scalar(out=eq0[:], in0=e[:], scalar1=m0v[:], scalar2=None, op0=alu.is_equal)
            sel0 = sm.tile([1, N_BINS], f32, tag="pk", bufs=5)
            nc.vector.select(out=sel0[:], mask=eq0[:], on_true=iota, on_false=bigt[:])
            idx0 = sm.tile([1, 1], f32)
            nc.vector.tensor_reduce(idx0[:], sel0[:], axis=mybir.AxisListType.X, op=alu.min)
            g0 = sm.tile([1, 1], dt.uint8)
            nc.vector.tensor_single_scalar(out=g0[:], in_=npk[:], scalar=0.0, op=alu.is_gt)
            g1 = sm.tile([1, 1], dt.uint8)
            nc.vector.tensor_single_scalar(out=g1[:], in_=npk[:], scalar=1.0, op=alu.is_gt)
            p1 = sm.tile([1, 1], f32)
            nc.vector.select(out=p1[:], mask=g0[:], on_true=idx1[:], on_false=idx0[:])
            p2 = sm.tile([1, 1], f32)
            nc.vector.select(out=p2[:], mask=g1[:], on_true=idx2[:], on_false=p1[:])
            a12 = sm.tile([1, 2], f32)
            oh = sm.tile([1, N_BINS], f32, tag="pk", bufs=5)
            nc.vector.tensor_scalar(out=oh[:], in0=iota, scalar1=p1[:], scalar2=None, op0=alu.is_equal)
            nc.vector.tensor_tensor(out=oh[:], in0=oh[:], in1=cent, op=alu.mult)
            nc.vector.reduce_sum(a12[:, 0:1], oh[:], axis=mybir.AxisListType.X)
            oh2 = sm.tile([1, N_BINS], f32, tag="pk", bufs=5)
            nc.vector.tensor_scalar(out=oh2[:], in0=iota, scalar1=p2[:], scalar2=None, op0=alu.is_equal)
            nc.vector.tensor_tensor(out=oh2[:], in0=oh2[:], in1=cent, op=alu.mult)
            nc.vector.reduce_sum(a12[:, 1:2], oh2[:], axis=mybir.AxisListType.X)

            # broadcast the two peak angles to all 128 partitions
            pb = ps1.tile([128, 2], f32, tag="pb", bufs=1)
            nc.tensor.matmul(pb[:], lhsT=ones_r[:], rhs=a12[:], start=True, stop=True)
            a12b = sm.tile([128, 2], f32)
            nc.vector.tensor_copy(out=a12b[:], in_=pb[:])

            # ---- gain map (transposed orientation) ----
            Eacc = sm.tile([128, 128], f32)
            dtl = sm.tile([128, 128], f32)
            ctmp = sm.tile([128, 128], f32)
            for pi in range(2):
                nc.vector.tensor_scalar(out=dtl[:], in0=thetaT, scalar1=a12b[:, pi:pi + 1],
                                        scalar2=None, op0=alu.subtract)
                nc.scalar.activation(out=dtl[:], in_=dtl[:], func=act.Abs)
                nc.vector.tensor_single_scalar(out=ctmp[:], in_=dtl[:], scalar=f32BW, op=alu.is_le)
                if pi == 0:
                    nc.vector.tensor_scalar(out=Eacc[:], in0=dtl[:], scalar1=f32PI, scalar2=-f32BW,
                                            op0=alu.subtract, op1=alu.is_ge)
                    nc.vector.tensor_tensor(out=Eacc[:], in0=Eacc[:], in1=ctmp[:], op=alu.add)
                else:
                    nc.vector.tensor_tensor(out=Eacc[:], in0=Eacc[:], in1=ctmp[:], op=alu.add)
                    nc.vector.tensor_scalar(out=ctmp[:], in0=dtl[:], scalar1=f32PI, scalar2=-f32BW,
                                            op0=alu.subtract, op1=alu.is_ge)
                    nc.vector.tensor_tensor(out=Eacc[:], in0=Eacc[:], in1=ctmp[:], op=alu.add)
            menh = sm.tile([128, 128], f32)
            nc.vector.tensor_single_scalar(out=menh[:], in_=Eacc[:], scalar=0.0, op=alu.is_gt)
            ghb = sm.tile([128, 128], f32)
            nc.vector.tensor_scalar(out=ghb[:], in0=menh[:], scalar1=ALPHA - BETA, scalar2=BETA,
                                    op0=alu.mult, op1=alu.add)
            gT = sm.tile([128, 128], f32)
            nc.vector.select(out=gT[:], mask=highT[:], on_true=ghb[:], on_false=one_m[:])

            # ---- gain mult + inverse DFT per plane; accumulate planes in SBUF ----
            xe_acc = sm.tile([128, 2048], f32)
            for m in range(16):
                ft = ft_tiles[m]
                gt2 = wk.tile([128, 256], f32r, tag="gt2")
                geng = nc.gpsimd if m % 2 == 0 else nc.vector
                geng.tensor_tensor(out=gt2[:, 0:128], in0=ft[:, 0:128], in1=gT[:], op=alu.mult)
                geng.tensor_tensor(out=gt2[:, 128:256], in0=ft[:, 128:256], in1=gT[:], op=alu.mult)
                ma = ps.tile([128, 256], f32, tag="mm")
                nc.tensor.matmul(ma[:], lhsT=gt2[:, 0:128], rhs=rhs_ci, start=True, stop=True)
                mb = ps.tile([128, 256], f32, tag="mm")
                nc.tensor.matmul(mb[:], lhsT=gt2[:, 128:256], rhs=rhs_ci, start=True, stop=True)
                mbs = wk.tile([128, 256], f32, tag="mbs")
                nc.vector.tensor_copy(out=mbs[:], in_=mb[:])
                u = wk.tile([128, 256], f32r, tag="u")
                nc.vector.tensor_tensor(out=u[:, 0:128], in0=ma[:, 0:128],
                                        in1=mbs[:, 128:256], op=alu.subtract)
                nc.vector.tensor_tensor(out=u[:, 128:256], in0=ma[:, 128:256],
                                        in1=mbs[:, 0:128], op=alu.add)
                m2a = ps.tile([128, 256], f32, tag="mm")
                nc.tensor.matmul(m2a[:], lhsT=cirT, rhs=u[:], start=True, stop=True)
                m2b = ps.tile([128, 256], f32, tag="mm")
                nc.tensor.matmul(m2b[:], lhsT=ciiT, rhs=u[:], start=True, stop=True)
                xbs = wk.tile([128, 128], f32, tag="xbs")
                nc.vector.tensor_copy(out=xbs[:], in_=m2b[:, 128:256])
                nc.vector.tensor_tensor(out=xe_acc[:, m * 128:(m + 1) * 128],
                                        in0=m2a[:, 0:128], in1=xbs[:], op=alu.subtract)
                if m == 7:
                    nc.sync.dma_start(
                        out=xe[0:8, :].rearrange("m (h w) -> h m w", h=128),
                        in_=xe_acc[:, 0:1024].rearrange("h (m w) -> h m w", m=8))
            nc.sync.dma_start(
                out=xe[8:16, :].rearrange("m (h w) -> h m w", h=128),
                in_=xe_acc[:, 1024:2048].rearrange("h (m w) -> h m w", m=8))
            # ---- proj_out + residual; 8 output DMAs of 1MB ----
            for g in range(8):
                xeg = wk.tile([16, 2048], f32r, tag="xeg", bufs=2)
                nc.sync.dma_start(out=xeg[:], in_=xe[:, g * 2048:(g + 1) * 2048].bitcast(f32r))
                osb = wk.tile([128, 2048], f32, tag="osb", bufs=2)
                for s in range(4):
                    c = g * 2048 + s * 512
                    pO = ps.tile([128, 512], f32, tag="mm")
                    nc.tensor.matmul(pO[:], lhsT=woutT[:], rhs=xeg[:, s * 512:(s + 1) * 512],
                                     start=True, stop=True)
                    nc.vector.tensor_tensor(out=osb[:, s * 512:(s + 1) * 512], in0=pO[:],
                                            in1=xh[:, c:c + 512].bitcast(f32), op=alu.add)
                if g < 7:
                    eng = nc.sync if g % 2 == 0 else nc.scalar
                    eng.dma_start(out=out_d[:, g * 2048:(g + 1) * 2048], in_=osb[:])
                else:
                    nc.sync.dma_start(out=out_d[:, g * 2048:g * 2048 + 1024], in_=osb[:, 0:1024])
                    nc.scalar.dma_start(out=out_d[:, g * 2048 + 1024:(g + 1) * 2048], in_=osb[:, 1024:2048])

    nc.finalize()
    return nc


def _get_compiled():
    if 'nc' not in _CACHE:
        _CACHE['nc'] = _build_graph()
        _CACHE['tables'] = _build_tables()
    return _CACHE['nc'], _CACHE['tables']


def run(x, w_in, w_out, trace=False):
    try:
        import jax
        jax.config.update('jax_compilation_cache_dir', '/tmp/angle_freq_jax_cache')
        jax.config.update('jax_persistent_cache_min_entry_size_bytes', -1)
        jax.config.update('jax_persistent_cache_min_compile_time_secs', 0.5)
    except Exception:
        pass
    from concourse.bass_utils import run_bass_kernel_spmd
    nc, T = _get_compiled()
    B, C, H, W = x.shape
    in_maps = []
    for i in range(8):
        b, hf = i // 2, i % 2
        m = dict(T)
        m['xh'] = np.ascontiguousarray(x[b, hf * 128:(hf + 1) * 128].reshape(128, 16384))
        m['xo'] = np.ascontiguousarray(x[b, (1 - hf) * 128:(2 - hf) * 128].reshape(128, 16384))
        m['winT'] = np.ascontiguousarray(w_in[:, hf * 128:(hf + 1) * 128].T)
        m['winO'] = np.ascontiguousarray(w_in[:, (1 - hf) * 128:(2 - hf) * 128].T)
        m['woutT'] = np.ascontiguousarray(w_out[hf * 128:(hf + 1) * 128, :].T)
        in_maps.append(m)
    res = run_bass_kernel_spmd(nc, in_maps, core_ids=list(range(8)), trace=trace)
    out = np.empty((B, C, H, W), dtype=np.float32)
    for i in range(8):
        b, hf = i // 2, i % 2
        out[b, hf * 128:(hf + 1) * 128] = res.results[i]['out'].reshape(128, H, W)
    return out, res


def kernel(x, w_in, w_out):
    out, _ = run(np.asarray(x, dtype=np.float32), np.asarray(w_in, dtype=np.float32),
                 np.asarray(w_out, dtype=np.float32))
    return out
